# revision 56
# baseline (speedup 1.0000x reference)
# Trainium2 Bass kernel for AttentionPooling (segment softmax-pool).
#
# Math: reference's per-slot max subtraction cancels in the softmax, so
#   w[t,k] = exp(s_t) / D_k,  D_k = sum_{t in slot_k} exp(s_t)
#   out[k,:] = sum_{t in slot_k} exp(s_t) * proj[t,:] / D_k
# (b2 shifts every score equally so it cancels too and is dropped.)
# With A[t,k] = in_slot(t,k) * exp(s_t), both numerator and D come from one
# accumulated PE matmul per 128-row chunk:  [num | D] += A^T @ [proj | 1].
#
# The score MLP needs proj with H on partitions; rather than transposing on
# the PE (costly: PE transpose + PSUM->SBUF copy per chunk), the host ships
# proj twice: t-major in fp16 (the output-forming segment matmul needs the
# precision) and h-major in fp8e4m3 (score errors only perturb softmax
# weights slightly; measured end-to-end rel err ~5e-3 vs the 2e-2 gate).
# That is 3 bytes/element of HBM traffic vs 4 for bf16-twice.
#
# Mask generation uses int16 boundaries + fp16 A so the DVE runs its ops in
# 2x_1p mode (all operands 2-byte).
#
# Software pipelining: per job, the h-major slab for job j+1 is loaded before
# the t-major data of job j, so exp-weights are ready when segment data lands
# and the post-DMA tail is only the last segment chain. The final job tapers
# (4/2/1/1 chunks) to keep that tail short.
#
# Sharding: data-parallel over B; core i handles batches 2i, 2i+1.

import numpy as np
import ml_dtypes

import concourse.bacc as bacc
import concourse.tile as tile
import concourse.mybir as mybir
import concourse.bass as bass
from concourse.bass_utils import run_bass_kernel_spmd

B, T, H, K = 16, 8192, 256, 128
HQ = 64
NCORES = 8
BPC = B // NCORES          # batches per core
CH = 128                   # rows per chunk
NCH = T // CH              # 64 chunks per batch
GRP = 8                    # chunks per DMA job
SUB = 8                    # chunks per W1-matmul / exp subgroup

F32 = mybir.dt.float32
F16 = mybir.dt.float16
FP8 = mybir.dt.float8e4
I16 = mybir.dt.int16


def make_jobs():
    jobs = []
    for b in range(BPC):
        for G in range(NCH // GRP):
            jobs.append((b, G * GRP, GRP))
    # taper the global tail so the last segment chain is short
    b, c0, n = jobs.pop()
    jobs += [(b, c0, 4), (b, c0 + 4, 2), (b, c0 + 6, 1), (b, c0 + 7, 1)]
    return jobs


def build_program():
    nc = bacc.Bacc(None, target_bir_lowering=False, debug=False)

    # t-major fp16 proj, host-tiled [b, G, p, g, h] so each partition reads
    # one contiguous 4KB run per job (DMA descriptor efficiency)
    NG = NCH // GRP
    proj = nc.dram_tensor("proj", [BPC, NG, CH, GRP, H], F16, kind="ExternalInput")
    # h-major fp8e4m3 proj (rhs of score matmul): [b, half, h_in_half, t]
    projt = nc.dram_tensor("projt", [BPC, 2, CH, T], FP8, kind="ExternalInput")
    # starts and ends packed: [2, b, k] as int16 (exact integer compares)
    bounds = nc.dram_tensor("bounds", [2, BPC, K], I16, kind="ExternalInput")
    # W1 halves: [half, p, hq] in fp8e4m3 (matches the fp8 score rhs; both
    # matmul operands must share a dtype class for the ISA)
    wpack = nc.dram_tensor("wpack", [2, CH, HQ], FP8, kind="ExternalInput")
    w2in = nc.dram_tensor("w2", [HQ], F16, kind="ExternalInput")
    b1 = nc.dram_tensor("b1", [HQ], F32, kind="ExternalInput")
    # tcol[p, c] = p + 128*c (t coordinate of row p in chunk c); f32 because
    # compare-op scalars must be f32 (values <= 8191 are exact)
    tcolin = nc.dram_tensor("tcol", [CH, NCH], F32, kind="ExternalInput")
    # raw [num | den] per slot, straight from PSUM; the final (tiny) divide
    # happens on the host so the device tail is one DMA shorter
    out = nc.dram_tensor("out", [BPC, K, H + 2], F32, kind="ExternalOutput")

    with tile.TileContext(nc) as tc:
        with (
            tc.tile_pool(name="const", bufs=1) as const,
            tc.tile_pool(name="projg", bufs=20) as projp,
            tc.tile_pool(name="projtg", bufs=8) as ptp,
            tc.tile_pool(name="htanh", bufs=4) as htp,
            tc.tile_pool(name="a1s", bufs=8) as a1pool,
            tc.tile_pool(name="amask", bufs=32) as apool,
            tc.tile_pool(name="eall", bufs=2) as epool,
            tc.tile_pool(name="outs", bufs=2) as outp,
            tc.tile_pool(name="misc", bufs=2) as miscp,
            tc.tile_pool(name="psH", bufs=2, space="PSUM") as psH,
            tc.tile_pool(name="psS", bufs=2, space="PSUM") as psS,
            tc.tile_pool(name="psSeg", bufs=2, space="PSUM") as psSeg,
        ):
            jobs = make_jobs()
            last_issued = {}
            for b_, c0_, n_ in jobs:
                last_issued[b_] = c0_ + n_ - 1
            e_alls = [
                epool.tile([CH, NCH], F32, tag="eall", name=f"e_all{b}")
                for b in range(BPC)
            ]
            segs = [
                psSeg.tile([K, H + 2], F32, tag="seg", name=f"seg{b}")
                for b in range(BPC)
            ]

            def dma_pt(b, c0, n):
                pt_tile = ptp.tile([CH, 2, GRP * CH], FP8, tag="pt")
                nc.sync.dma_start(
                    out=pt_tile[:, :, 0 : n * CH],
                    in_=bass.AP(
                        projt,
                        b * 2 * CH * T + c0 * CH,
                        [[T, CH], [CH * T, 2], [1, n * CH]],
                    ),
                )
                return pt_tile

            def dma_pt_pair(b, c0, split=None):
                # one 2-job slab: bigger transfers keep the DMA engines fed
                # while the shared HWDGE churns through early instructions.
                # split=(2, 6) additionally lands the first two chunks as a
                # separate transfer so the first exp/mask chain starts ~2us
                # sooner.
                w = 2 * GRP * CH
                pt_tile = ptp.tile([CH, 2, w], FP8, tag="ptw", bufs=8)
                src_off = b * 2 * CH * T + c0 * CH
                if split:
                    cut = split[0] * CH
                    nc.sync.dma_start(
                        out=pt_tile[:, :, 0:cut],
                        in_=bass.AP(
                            projt, src_off, [[T, CH], [CH * T, 2], [1, cut]]
                        ),
                    )
                    nc.sync.dma_start(
                        out=pt_tile[:, :, cut:w],
                        in_=bass.AP(
                            projt,
                            src_off + cut,
                            [[T, CH], [CH * T, 2], [1, w - cut]],
                        ),
                    )
                else:
                    nc.sync.dma_start(
                        out=pt_tile[:, :, 0:w],
                        in_=bass.AP(
                            projt, src_off, [[T, CH], [CH * T, 2], [1, w]]
                        ),
                    )
                return pt_tile

            # ---- constants. The score-path consts ride the Act HWDGE queue
            # in dependency order (wp for the first matmul, then b1 for the
            # first tanh, then w2); tcol/bnd go through the separate GPSIMD
            # SWDGE path so they don't steal HWDGE slots from the pt stream.
            wp = const.tile([CH, 2, HQ], FP8)
            nc.scalar.dma_start(
                out=wp[:],
                in_=bass.AP(wpack, 0, [[HQ, CH], [CH * HQ, 2], [1, HQ]]),
            )
            w2_sb = const.tile([HQ, 1], F16)
            nc.scalar.dma_start(out=w2_sb[:], in_=bass.AP(w2in, 0, [[1, HQ], [1, 1]]))
            b1_sb = const.tile([HQ, 1], F32)
            nc.gpsimd.dma_start(out=b1_sb[:], in_=bass.AP(b1, 0, [[1, HQ], [1, 1]]))
            tcol = const.tile([CH, NCH], F32)
            nc.gpsimd.dma_start(
                out=tcol[:], in_=bass.AP(tcolin, 0, [[NCH, CH], [1, NCH]])
            )
            # boundaries broadcast down all 128 partitions: [p, se, b, k]
            bnd = const.tile([CH, 2, BPC, K], I16)
            nc.gpsimd.dma_start(
                out=bnd[:],
                in_=bass.AP(bounds, 0, [[0, CH], [BPC * K, 2], [K, BPC], [1, K]]),
            )

            def scores(b, c0, n, pt_tile, off=0, groups=None):
                e_all = e_alls[b]
                if groups is None:
                    groups = [(i, min(SUB, n - i)) for i in range(0, n, SUB)]
                for s0, ns in groups:
                    s_ps = psS.tile([CH, SUB], F32, tag="sps")
                    hps = psH.tile([HQ, SUB, CH], F32, tag="hps")
                    # a single matmul may write at most 512 f32/partition of
                    # PSUM (one bank), so emit the 8-chunk group as two halves
                    for q0 in range(0, ns, 4):
                        nq = min(4, ns - q0)
                        for half in range(2):
                            nc.tensor.matmul(
                                hps[:, q0 : q0 + nq, :],
                                wp[:, half, :],
                                pt_tile[
                                    :,
                                    half,
                                    off + (s0 + q0) * CH
                                    : off + (s0 + q0 + nq) * CH,
                                ],
                                start=(half == 0),
                                stop=(half == 1),
                            )
                    hts = htp.tile([HQ, SUB, CH], F16, tag="hts")
                    nc.scalar.activation(
                        out=hts[:, 0:ns, :],
                        in_=hps[:, 0:ns, :],
                        func=mybir.ActivationFunctionType.Tanh,
                        bias=b1_sb[:],
                        scale=1.0,
                    )
                    for j in range(ns):
                        nc.tensor.matmul(
                            s_ps[:, j : j + 1],
                            hts[:, j, :],
                            w2_sb[:],
                            start=True,
                            stop=True,
                        )
                    nc.scalar.activation(
                        out=e_all[:, c0 + s0 : c0 + s0 + ns],
                        in_=s_ps[:, 0:ns],
                        func=mybir.ActivationFunctionType.Exp,
                    )

            def agen(b, c0, n):
                # a1 runs in DVE 4x mode (all 2-byte operands); a2 has two
                # tensor inputs so no fast mode exists -> alternate it between
                # DVE and GPSIMD to balance the two queues.
                e_all = e_alls[b]
                a2s = []
                for g in range(n):
                    c = c0 + g
                    a1 = a1pool.tile([CH, K], F16, tag="a1")
                    a2 = apool.tile([CH, K], F16, tag="a2")
                    # a1[t,k] = (start_k <= t) * E_t; alternate engines --
                    # GPSIMD cannot run the two-tensor-input a2 form, so it
                    # takes half the a1 ops instead
                    a1_eng = nc.vector if (c % 3 == 0) else nc.gpsimd
                    a1_eng.tensor_scalar(
                        out=a1[:],
                        in0=bnd[:, 0, b, :],
                        scalar1=tcol[:, c : c + 1],
                        scalar2=e_all[:, c : c + 1],
                        op0=mybir.AluOpType.is_le,
                        op1=mybir.AluOpType.mult,
                    )
                    # a2[t,k] = (end_k > t) * a1
                    nc.vector.scalar_tensor_tensor(
                        out=a2[:],
                        in0=bnd[:, 1, b, :],
                        scalar=tcol[:, c : c + 1],
                        in1=a1[:],
                        op0=mybir.AluOpType.is_gt,
                        op1=mybir.AluOpType.mult,
                    )
                    a2s.append(a2)
                return a2s

            def dma_g(b, c0, n, eng=None):
                g_tile = projp.tile([CH, GRP, H + 2], F16, tag="g")
                G, g0 = c0 // GRP, c0 % GRP
                (eng or nc.sync).dma_start(
                    out=g_tile[:, 0:n, 0:H],
                    in_=bass.AP(
                        proj,
                        (b * (NCH // GRP) + G) * CH * GRP * H + g0 * H,
                        [[GRP * H, CH], [H, n], [1, H]],
                    ),
                )
                nc.gpsimd.memset(g_tile[:, 0:n, H : H + 2], 1.0)
                return g_tile

            def seg_group(b, c0, n, a2s, g_tile):
                seg = segs[b]
                for g in range(n):
                    c = c0 + g
                    nc.tensor.matmul(
                        seg[:],
                        a2s[g][:],
                        g_tile[:, g, :],
                        start=(c == 0),
                        stop=(c == last_issued[b]),
                    )

            def epilogue(b):
                # PSUM->SBUF copy split across two engines in parallel, and
                # batch 1's out-DMA rides the (by-then idle) SP queue whose
                # DGE delay is 134ns shorter than Act's -- this chain is the
                # kernel's absolute tail
                seg = segs[b]
                ot = outp.tile([K, H + 2], F16)
                nc.scalar.copy(out=ot[:], in_=seg[:])
                eng = nc.sync if b == 1 else nc.scalar
                eng.dma_start(
                    out=bass.AP(out, b * K * (H + 2), [[H + 2, K], [1, H + 2]]),
                    in_=ot[:],
                )

            # Every job owns its tiles, so the SP DMA stream below is fully
            # wait-free: the DMA engines run back-to-back transfers while the
            # compute queues chase the arrivals via semaphores. The taper
            # jobs' (tiny) score slabs load first so the end-of-kernel tail
            # is only: last t-major load -> one matmul -> epilogue.
            last_jx = {}
            for jx, (b_, c0_, n_) in enumerate(jobs):
                last_jx[b_] = jx
            ntaper = 4
            taper_ids = list(range(len(jobs) - ntaper, len(jobs)))
            regular = [j for j in range(len(jobs)) if j not in taper_ids]
            # two big slabs first (the DMA engines outpace the 650ns/instr
            # SP issue rate on tiny transfers), then the tiny taper slabs,
            # then the rest with a 6-job lead over the t-major stream so
            # every score/mask chain finishes long before its seg data lands
            # taper slabs are tiny and only needed late; slot them mid-
            # stream so the first transfers are all full-size
            pt_order = regular[:10] + taper_ids + regular[10:]
            pt_tiles = {}
            pt_offs = {}
            score_groups = {}
            g_tiles = {}

            def issue_pt(item):
                kind, j = item
                if kind == "p":
                    tile = dma_pt_pair(jobs[j][0], jobs[j][1])
                    for k in (j, j + 1):
                        pt_tiles[k] = tile
                        pt_offs[k] = (k - j) * GRP * CH
                else:
                    pt_tiles[j] = dma_pt(*jobs[j])

            # consecutive same-batch jobs share one double slab: half the
            # early DMA instructions, so the engines never outrun the 650ns
            # per-instruction issue rate
            # job 0's slab stays single so the first exp/mask chain starts
            # ~0.7us sooner; the pairs shift to jobs (1,2) and (3,4)
            pt_items = (
                [("s", 0), ("p", 1), ("p", 3)]
                + [("s", j) for j in (5, 6, 7, 8, 9)]
                + [("s", t) for t in taper_ids]
                + [("s", j) for j in (10, 11, 12, 13, 14)]
            )
            NUP = 8
            for it in pt_items[:NUP]:
                issue_pt(it)
            rest = pt_items[NUP:]
            for jx in range(len(jobs)):
                eng = nc.scalar if jx >= len(jobs) - 5 else None
                g_tiles[jx] = dma_g(*jobs[jx], eng=eng)
                if rest:
                    issue_pt(rest.pop(0))

            # the taper jobs' scores/masks are computed mid-kernel (once
            # their slabs have landed) so the in-order PE/Act/DVE queues
            # don't leave them until after every regular seg matmul; the
            # post-DMA tail is then just their seg matmuls + epilogue
            a2_map = {}
            HOIST = 10
            EPI0 = 11
            for jx, (b, c0, n) in enumerate(jobs):
                if jx in a2_map:
                    seg_group(b, c0, n, a2_map.pop(jx), g_tiles.pop(jx))
                else:
                    scores(b, c0, n, pt_tiles.pop(jx),
                           off=pt_offs.get(jx, 0),
                           groups=score_groups.get(jx))
                    a2s = agen(b, c0, n)
                    seg_group(b, c0, n, a2s, g_tiles.pop(jx))
                if jx == HOIST:
                    for tx in taper_ids:
                        scores(*jobs[tx], pt_tiles.pop(tx))
                        a2_map[tx] = agen(*jobs[tx])
                # a DMA instruction holds its sequencer through its waits, so
                # issue batch 0's out-DMA only once its seg PSUM is nearly
                # complete -- issuing it at jx==last_jx[0] would block the
                # Act queue (and every later tanh/exp) for several us
                if jx == EPI0:
                    epilogue(0)
                if jx == len(jobs) - 1:
                    epilogue(1)

    nc.compile()
    return nc


_prog_cache = None
LAST_RESULTS = None


def _get_program():
    global _prog_cache
    if _prog_cache is None:
        _prog_cache = build_program()
    return _prog_cache


def kernel(**inputs):
    proj = np.asarray(inputs["projected"], dtype=np.float32)
    bnds = np.asarray(inputs["boundaries"])
    slot = np.asarray(inputs["slot_mask"])
    W1 = np.asarray(inputs["W1"], dtype=np.float32)
    b1 = np.ascontiguousarray(np.asarray(inputs["b1"], dtype=np.float32))
    W2 = np.asarray(inputs["W2"], dtype=np.float32).reshape(HQ)

    live = slot > 0
    starts = np.where(live, bnds[..., 0], 0).astype(np.int16)   # [B, K]
    ends = np.where(live, bnds[..., 1], 0).astype(np.int16)

    projt_8 = np.ascontiguousarray(
        proj.transpose(0, 2, 1).reshape(B, 2, CH, T)
    ).astype(ml_dtypes.float8_e4m3)                               # [B, 2, 128, T]
    # [B, T, H] -> [B, G, p, g, h]: per-partition contiguous job runs
    proj_16 = np.ascontiguousarray(
        proj.astype(np.float16)
        .reshape(B, NCH // GRP, GRP, CH, H)
        .transpose(0, 1, 3, 2, 4)
    )

    wpack = np.ascontiguousarray(
        W1.reshape(2, CH, HQ).astype(ml_dtypes.float8_e4m3)
    )
    w2_16 = W2.astype(np.float16)

    tcol = (np.arange(CH)[:, None] + CH * np.arange(NCH)[None, :]).astype(
        np.float32
    )

    nc = _get_program()
    in_maps = []
    for i in range(NCORES):
        lo, hi = i * BPC, (i + 1) * BPC
        in_maps.append(
            {
                "proj": proj_16[lo:hi],
                "projt": projt_8[lo:hi],
                "bounds": np.ascontiguousarray(
                    np.stack([starts[lo:hi], ends[lo:hi]])
                ),
                "wpack": wpack,
                "w2": w2_16,
                "b1": b1,
                "tcol": tcol,
            }
        )

    res = run_bass_kernel_spmd(nc, in_maps, core_ids=list(range(NCORES)))
    global LAST_RESULTS
    LAST_RESULTS = res
    outs = np.concatenate([r["out"] for r in res.results], axis=0)
    raw = outs.reshape(B, K, H + 2).astype(np.float32)
    den = raw[:, :, H : H + 1]
    return (raw[:, :, 0:H] / np.where(den > 0, den, 1.0)).astype(np.float32)


# revision 58
# speedup vs baseline: 1.0018x; 1.0018x over previous
# Trainium2 Bass kernel for AttentionPooling (segment softmax-pool).
#
# Math: reference's per-slot max subtraction cancels in the softmax, so
#   w[t,k] = exp(s_t) / D_k,  D_k = sum_{t in slot_k} exp(s_t)
#   out[k,:] = sum_{t in slot_k} exp(s_t) * proj[t,:] / D_k
# (b2 shifts every score equally so it cancels too and is dropped.)
# With A[t,k] = in_slot(t,k) * exp(s_t), both numerator and D come from one
# accumulated PE matmul per 128-row chunk:  [num | D] += A^T @ [proj | 1].
#
# The score MLP needs proj with H on partitions; rather than transposing on
# the PE (costly: PE transpose + PSUM->SBUF copy per chunk), the host ships
# proj twice: t-major in fp16 (the output-forming segment matmul needs the
# precision) and h-major in fp8e4m3 (score errors only perturb softmax
# weights slightly; measured end-to-end rel err ~5e-3 vs the 2e-2 gate).
# That is 3 bytes/element of HBM traffic vs 4 for bf16-twice.
#
# Mask generation uses int16 boundaries + fp16 A so the DVE runs its ops in
# 2x_1p mode (all operands 2-byte).
#
# Software pipelining: per job, the h-major slab for job j+1 is loaded before
# the t-major data of job j, so exp-weights are ready when segment data lands
# and the post-DMA tail is only the last segment chain. The final job tapers
# (4/2/1/1 chunks) to keep that tail short.
#
# Sharding: data-parallel over B; core i handles batches 2i, 2i+1.

import numpy as np
import ml_dtypes

import concourse.bacc as bacc
import concourse.tile as tile
import concourse.mybir as mybir
import concourse.bass as bass
from concourse.bass_utils import run_bass_kernel_spmd

B, T, H, K = 16, 8192, 256, 128
HQ = 64
NCORES = 8
BPC = B // NCORES          # batches per core
CH = 128                   # rows per chunk
NCH = T // CH              # 64 chunks per batch
GRP = 8                    # chunks per DMA job
SUB = 8                    # chunks per W1-matmul / exp subgroup

F32 = mybir.dt.float32
F16 = mybir.dt.float16
FP8 = mybir.dt.float8e4
I16 = mybir.dt.int16


def make_jobs():
    jobs = []
    for b in range(BPC):
        for G in range(NCH // GRP):
            jobs.append((b, G * GRP, GRP))
    # taper the global tail so the last segment chain is short
    b, c0, n = jobs.pop()
    jobs += [(b, c0, 4), (b, c0 + 4, 2), (b, c0 + 6, 1), (b, c0 + 7, 1)]
    return jobs


def build_program():
    nc = bacc.Bacc(None, target_bir_lowering=False, debug=False)

    # t-major fp16 proj, host-tiled [b, G, p, g, h] so each partition reads
    # one contiguous 4KB run per job (DMA descriptor efficiency)
    NG = NCH // GRP
    proj = nc.dram_tensor("proj", [BPC, NG, CH, GRP, H], F16, kind="ExternalInput")
    # h-major fp8e4m3 proj (rhs of score matmul): [b, half, h_in_half, t]
    projt = nc.dram_tensor("projt", [BPC, 2, CH, T], FP8, kind="ExternalInput")
    # starts and ends packed: [2, b, k] as int16 (exact integer compares)
    bounds = nc.dram_tensor("bounds", [2, BPC, K], I16, kind="ExternalInput")
    # W1 halves: [half, p, hq] in fp8e4m3 (matches the fp8 score rhs; both
    # matmul operands must share a dtype class for the ISA)
    wpack = nc.dram_tensor("wpack", [2, CH, HQ], FP8, kind="ExternalInput")
    w2in = nc.dram_tensor("w2", [HQ], F16, kind="ExternalInput")
    b1 = nc.dram_tensor("b1", [HQ], F32, kind="ExternalInput")
    # tcol[p, c] = p + 128*c (t coordinate of row p in chunk c); f32 because
    # compare-op scalars must be f32 (values <= 8191 are exact)
    tcolin = nc.dram_tensor("tcol", [CH, NCH], F32, kind="ExternalInput")
    # raw [num | den] per slot, straight from PSUM; the final (tiny) divide
    # happens on the host so the device tail is one DMA shorter
    out = nc.dram_tensor("out", [BPC, K, H + 2], F32, kind="ExternalOutput")

    with tile.TileContext(nc) as tc:
        with (
            tc.tile_pool(name="const", bufs=1) as const,
            tc.tile_pool(name="projg", bufs=20) as projp,
            tc.tile_pool(name="projtg", bufs=8) as ptp,
            tc.tile_pool(name="htanh", bufs=4) as htp,
            tc.tile_pool(name="a1s", bufs=8) as a1pool,
            tc.tile_pool(name="amask", bufs=56) as apool,
            tc.tile_pool(name="eall", bufs=2) as epool,
            tc.tile_pool(name="outs", bufs=2) as outp,
            tc.tile_pool(name="misc", bufs=2) as miscp,
            tc.tile_pool(name="psH", bufs=2, space="PSUM") as psH,
            tc.tile_pool(name="psS", bufs=2, space="PSUM") as psS,
            tc.tile_pool(name="psSeg", bufs=2, space="PSUM") as psSeg,
        ):
            jobs = make_jobs()
            last_issued = {}
            for b_, c0_, n_ in jobs:
                last_issued[b_] = c0_ + n_ - 1
            e_alls = [
                epool.tile([CH, NCH], F32, tag="eall", name=f"e_all{b}")
                for b in range(BPC)
            ]
            segs = [
                psSeg.tile([K, H + 2], F32, tag="seg", name=f"seg{b}")
                for b in range(BPC)
            ]

            def dma_pt(b, c0, n):
                pt_tile = ptp.tile([CH, 2, GRP * CH], FP8, tag="pt")
                nc.sync.dma_start(
                    out=pt_tile[:, :, 0 : n * CH],
                    in_=bass.AP(
                        projt,
                        b * 2 * CH * T + c0 * CH,
                        [[T, CH], [CH * T, 2], [1, n * CH]],
                    ),
                )
                return pt_tile

            def dma_pt_pair(b, c0, split=None):
                # one 2-job slab: bigger transfers keep the DMA engines fed
                # while the shared HWDGE churns through early instructions.
                # split=(2, 6) additionally lands the first two chunks as a
                # separate transfer so the first exp/mask chain starts ~2us
                # sooner.
                w = 2 * GRP * CH
                pt_tile = ptp.tile([CH, 2, w], FP8, tag="ptw", bufs=8)
                src_off = b * 2 * CH * T + c0 * CH
                if split:
                    cut = split[0] * CH
                    nc.sync.dma_start(
                        out=pt_tile[:, :, 0:cut],
                        in_=bass.AP(
                            projt, src_off, [[T, CH], [CH * T, 2], [1, cut]]
                        ),
                    )
                    nc.sync.dma_start(
                        out=pt_tile[:, :, cut:w],
                        in_=bass.AP(
                            projt,
                            src_off + cut,
                            [[T, CH], [CH * T, 2], [1, w - cut]],
                        ),
                    )
                else:
                    nc.sync.dma_start(
                        out=pt_tile[:, :, 0:w],
                        in_=bass.AP(
                            projt, src_off, [[T, CH], [CH * T, 2], [1, w]]
                        ),
                    )
                return pt_tile

            # ---- constants. The score-path consts ride the Act HWDGE queue
            # in dependency order (wp for the first matmul, then b1 for the
            # first tanh, then w2); tcol/bnd go through the separate GPSIMD
            # SWDGE path so they don't steal HWDGE slots from the pt stream.
            wp = const.tile([CH, 2, HQ], FP8)
            nc.scalar.dma_start(
                out=wp[:],
                in_=bass.AP(wpack, 0, [[HQ, CH], [CH * HQ, 2], [1, HQ]]),
            )
            w2_sb = const.tile([HQ, 1], F16)
            nc.scalar.dma_start(out=w2_sb[:], in_=bass.AP(w2in, 0, [[1, HQ], [1, 1]]))
            b1_sb = const.tile([HQ, 1], F32)
            nc.gpsimd.dma_start(out=b1_sb[:], in_=bass.AP(b1, 0, [[1, HQ], [1, 1]]))
            tcol = const.tile([CH, NCH], F32)
            nc.gpsimd.dma_start(
                out=tcol[:], in_=bass.AP(tcolin, 0, [[NCH, CH], [1, NCH]])
            )
            # boundaries broadcast down all 128 partitions: [p, se, b, k]
            bnd = const.tile([CH, 2, BPC, K], I16)
            nc.gpsimd.dma_start(
                out=bnd[:],
                in_=bass.AP(bounds, 0, [[0, CH], [BPC * K, 2], [K, BPC], [1, K]]),
            )

            def scores(b, c0, n, pt_tile, off=0, groups=None):
                e_all = e_alls[b]
                if groups is None:
                    groups = [(i, min(SUB, n - i)) for i in range(0, n, SUB)]
                for s0, ns in groups:
                    s_ps = psS.tile([CH, SUB], F32, tag="sps")
                    hps = psH.tile([HQ, SUB, CH], F32, tag="hps")
                    # a single matmul may write at most 512 f32/partition of
                    # PSUM (one bank), so emit the 8-chunk group as two halves
                    for q0 in range(0, ns, 4):
                        nq = min(4, ns - q0)
                        for half in range(2):
                            nc.tensor.matmul(
                                hps[:, q0 : q0 + nq, :],
                                wp[:, half, :],
                                pt_tile[
                                    :,
                                    half,
                                    off + (s0 + q0) * CH
                                    : off + (s0 + q0 + nq) * CH,
                                ],
                                start=(half == 0),
                                stop=(half == 1),
                            )
                    hts = htp.tile([HQ, SUB, CH], F16, tag="hts")
                    nc.scalar.activation(
                        out=hts[:, 0:ns, :],
                        in_=hps[:, 0:ns, :],
                        func=mybir.ActivationFunctionType.Tanh,
                        bias=b1_sb[:],
                        scale=1.0,
                    )
                    for j in range(ns):
                        nc.tensor.matmul(
                            s_ps[:, j : j + 1],
                            hts[:, j, :],
                            w2_sb[:],
                            start=True,
                            stop=True,
                        )
                    nc.scalar.activation(
                        out=e_all[:, c0 + s0 : c0 + s0 + ns],
                        in_=s_ps[:, 0:ns],
                        func=mybir.ActivationFunctionType.Exp,
                    )

            def agen(b, c0, n):
                # a1 runs in DVE 4x mode (all 2-byte operands); a2 has two
                # tensor inputs so no fast mode exists -> alternate it between
                # DVE and GPSIMD to balance the two queues.
                e_all = e_alls[b]
                a2s = []
                for g in range(n):
                    c = c0 + g
                    a1 = a1pool.tile([CH, K], F16, tag="a1")
                    a2 = apool.tile([CH, K], F16, tag="a2")
                    # a1[t,k] = (start_k <= t) * E_t; alternate engines --
                    # GPSIMD cannot run the two-tensor-input a2 form, so it
                    # takes half the a1 ops instead
                    a1_eng = nc.vector if (c % 3 == 0) else nc.gpsimd
                    a1_eng.tensor_scalar(
                        out=a1[:],
                        in0=bnd[:, 0, b, :],
                        scalar1=tcol[:, c : c + 1],
                        scalar2=e_all[:, c : c + 1],
                        op0=mybir.AluOpType.is_le,
                        op1=mybir.AluOpType.mult,
                    )
                    # a2[t,k] = (end_k > t) * a1
                    nc.vector.scalar_tensor_tensor(
                        out=a2[:],
                        in0=bnd[:, 1, b, :],
                        scalar=tcol[:, c : c + 1],
                        in1=a1[:],
                        op0=mybir.AluOpType.is_gt,
                        op1=mybir.AluOpType.mult,
                    )
                    a2s.append(a2)
                return a2s

            def dma_g(b, c0, n, eng=None):
                g_tile = projp.tile([CH, GRP, H + 2], F16, tag="g")
                G, g0 = c0 // GRP, c0 % GRP
                (eng or nc.sync).dma_start(
                    out=g_tile[:, 0:n, 0:H],
                    in_=bass.AP(
                        proj,
                        (b * (NCH // GRP) + G) * CH * GRP * H + g0 * H,
                        [[GRP * H, CH], [H, n], [1, H]],
                    ),
                )
                nc.gpsimd.memset(g_tile[:, 0:n, H : H + 2], 1.0)
                return g_tile

            def seg_group(b, c0, n, a2s, g_tile):
                seg = segs[b]
                for g in range(n):
                    c = c0 + g
                    nc.tensor.matmul(
                        seg[:],
                        a2s[g][:],
                        g_tile[:, g, :],
                        start=(c == 0),
                        stop=(c == last_issued[b]),
                    )

            def epilogue(b):
                # PSUM->SBUF copy split across two engines in parallel, and
                # batch 1's out-DMA rides the (by-then idle) SP queue whose
                # DGE delay is 134ns shorter than Act's -- this chain is the
                # kernel's absolute tail
                seg = segs[b]
                ot = outp.tile([K, H + 2], F16)
                nc.scalar.copy(out=ot[:], in_=seg[:])
                eng = nc.sync if b == 1 else nc.scalar
                eng.dma_start(
                    out=bass.AP(out, b * K * (H + 2), [[H + 2, K], [1, H + 2]]),
                    in_=ot[:],
                )

            # Every job owns its tiles, so the SP DMA stream below is fully
            # wait-free: the DMA engines run back-to-back transfers while the
            # compute queues chase the arrivals via semaphores. The taper
            # jobs' (tiny) score slabs load first so the end-of-kernel tail
            # is only: last t-major load -> one matmul -> epilogue.
            last_jx = {}
            for jx, (b_, c0_, n_) in enumerate(jobs):
                last_jx[b_] = jx
            ntaper = 4
            taper_ids = list(range(len(jobs) - ntaper, len(jobs)))
            regular = [j for j in range(len(jobs)) if j not in taper_ids]
            # two big slabs first (the DMA engines outpace the 650ns/instr
            # SP issue rate on tiny transfers), then the tiny taper slabs,
            # then the rest with a 6-job lead over the t-major stream so
            # every score/mask chain finishes long before its seg data lands
            # taper slabs are tiny and only needed late; slot them mid-
            # stream so the first transfers are all full-size
            pt_order = regular[:10] + taper_ids + regular[10:]
            pt_tiles = {}
            pt_offs = {}
            score_groups = {}
            g_tiles = {}

            def issue_pt(item):
                kind, j = item
                if kind == "p":
                    tile = dma_pt_pair(jobs[j][0], jobs[j][1])
                    for k in (j, j + 1):
                        pt_tiles[k] = tile
                        pt_offs[k] = (k - j) * GRP * CH
                else:
                    pt_tiles[j] = dma_pt(*jobs[j])

            # consecutive same-batch jobs share one double slab: half the
            # early DMA instructions, so the engines never outrun the 650ns
            # per-instruction issue rate
            # job 0's slab stays single so the first exp/mask chain starts
            # ~0.7us sooner; the pairs shift to jobs (1,2) and (3,4)
            pt_items = (
                [("s", 0), ("p", 1), ("p", 3)]
                + [("s", j) for j in (5, 6, 7, 8, 9)]
                + [("s", t) for t in taper_ids]
                + [("s", j) for j in (10, 11, 12, 13, 14)]
            )
            NUP = 8
            for it in pt_items[:NUP]:
                issue_pt(it)
            rest = pt_items[NUP:]
            for jx in range(len(jobs)):
                eng = nc.scalar if jx >= len(jobs) - 6 else None
                g_tiles[jx] = dma_g(*jobs[jx], eng=eng)
                if rest:
                    issue_pt(rest.pop(0))

            # the taper jobs' scores/masks are computed mid-kernel (once
            # their slabs have landed) so the in-order PE/Act/DVE queues
            # don't leave them until after every regular seg matmul; the
            # post-DMA tail is then just their seg matmuls + epilogue
            a2_map = {}
            HOIST = 10
            EPI0 = 11
            for jx, (b, c0, n) in enumerate(jobs):
                if jx in a2_map:
                    seg_group(b, c0, n, a2_map.pop(jx), g_tiles.pop(jx))
                else:
                    scores(b, c0, n, pt_tiles.pop(jx),
                           off=pt_offs.get(jx, 0),
                           groups=score_groups.get(jx))
                    a2s = agen(b, c0, n)
                    seg_group(b, c0, n, a2s, g_tiles.pop(jx))
                if jx == HOIST:
                    for tx in taper_ids:
                        scores(*jobs[tx], pt_tiles.pop(tx))
                        a2_map[tx] = agen(*jobs[tx])
                # a DMA instruction holds its sequencer through its waits, so
                # issue batch 0's out-DMA only once its seg PSUM is nearly
                # complete -- issuing it at jx==last_jx[0] would block the
                # Act queue (and every later tanh/exp) for several us
                if jx == EPI0:
                    epilogue(0)
                if jx == len(jobs) - 1:
                    epilogue(1)

    nc.compile()
    return nc


_prog_cache = None
LAST_RESULTS = None


def _get_program():
    global _prog_cache
    if _prog_cache is None:
        _prog_cache = build_program()
    return _prog_cache


def kernel(**inputs):
    proj = np.asarray(inputs["projected"], dtype=np.float32)
    bnds = np.asarray(inputs["boundaries"])
    slot = np.asarray(inputs["slot_mask"])
    W1 = np.asarray(inputs["W1"], dtype=np.float32)
    b1 = np.ascontiguousarray(np.asarray(inputs["b1"], dtype=np.float32))
    W2 = np.asarray(inputs["W2"], dtype=np.float32).reshape(HQ)

    live = slot > 0
    starts = np.where(live, bnds[..., 0], 0).astype(np.int16)   # [B, K]
    ends = np.where(live, bnds[..., 1], 0).astype(np.int16)

    projt_8 = np.ascontiguousarray(
        proj.transpose(0, 2, 1).reshape(B, 2, CH, T)
    ).astype(ml_dtypes.float8_e4m3)                               # [B, 2, 128, T]
    # [B, T, H] -> [B, G, p, g, h]: per-partition contiguous job runs
    proj_16 = np.ascontiguousarray(
        proj.astype(np.float16)
        .reshape(B, NCH // GRP, GRP, CH, H)
        .transpose(0, 1, 3, 2, 4)
    )

    wpack = np.ascontiguousarray(
        W1.reshape(2, CH, HQ).astype(ml_dtypes.float8_e4m3)
    )
    w2_16 = W2.astype(np.float16)

    tcol = (np.arange(CH)[:, None] + CH * np.arange(NCH)[None, :]).astype(
        np.float32
    )

    nc = _get_program()
    in_maps = []
    for i in range(NCORES):
        lo, hi = i * BPC, (i + 1) * BPC
        in_maps.append(
            {
                "proj": proj_16[lo:hi],
                "projt": projt_8[lo:hi],
                "bounds": np.ascontiguousarray(
                    np.stack([starts[lo:hi], ends[lo:hi]])
                ),
                "wpack": wpack,
                "w2": w2_16,
                "b1": b1,
                "tcol": tcol,
            }
        )

    res = run_bass_kernel_spmd(nc, in_maps, core_ids=list(range(NCORES)))
    global LAST_RESULTS
    LAST_RESULTS = res
    outs = np.concatenate([r["out"] for r in res.results], axis=0)
    raw = outs.reshape(B, K, H + 2).astype(np.float32)
    den = raw[:, :, H : H + 1]
    return (raw[:, :, 0:H] / np.where(den > 0, den, 1.0)).astype(np.float32)
